# revision 4
# baseline (speedup 1.0000x reference)
"""Trainium2 Bass kernel for nn_Bottleneck_refine (grouped bottleneck + block mask).

Reference computation (per image b):
    m   = upsample(mask[b])            # [4,7,7] -> per-group 56x56 {0,1}
    t1  = conv1x1_g4(x * m1)           # 512 -> 128, but 1x1 commutes with mask
    a1  = m . relu(s1*t1 + c1)
    t2  = conv3x3_g4(a1)               # 128 -> 128 (pad 1)
    a2  = m . relu(s2*t2 + c2)
    y   = relu(s3*conv1x1_g4(a2) + c3 + x)

Identity used: for m in {0,1}:  m*relu(z) == relu(m*z), and the 1x1 conv
commutes with per-pixel masking, so the input mask multiply is absorbed.

Sharding: data-parallel over batch, 2 images per core on 8 cores.
All I/O and activations bf16 (host converts): ~6.4 MB in + 6.4 MB out
per core -> memory roofline ~36us at 358 GB/s.

Layouts per image (SBUF bf16 [partition, free]):
  xt      [128, G, 1568] per superchunk k (partition = in-channel of group)
  a1h     [128, 58, 60]  halo'd masked mid activation (2-col left pad for
                         4B-aligned DVE writes)
  a2      [128, 392] per (g, k): partition 32j+co = chunk 4k+j, mid-ch co
  chunks: 7 image rows (392 px); superchunk = 4 chunks = quad of PSUM banks.

PSUM: two 4-bank quad tiles (q0 = banks 0-3, q1 = banks 4-7) alternating
between stage A chunks-quads, stage B (bank=group), stage C (bank=chunk).
Elementwise ops read whole quads (1568 px per op) to amortize fixed costs.

Final stage relu(conv3 + b3 + x): two engine paths, balanced across
DVE / ACT / GPSIMD since only DVE can do the 3-operand PSUM op:
  path A: DVE stt (p3 + b3 + x) -> ot, DVE max0 in-place
  path B: ACT copy (p3 + b3) -> t, GPS add (t + x) -> ot, GPS max0

DMA plan: 2 const + 4 input + 16 tiny mask DMAs on the sync HWDGE ring,
4 output DMAs on the scalar ring (sequencer dispatch is ~600ns per DMA,
so few big transfers).
"""

import numpy as np

import concourse.bass as bass
import concourse.tile as tile
from concourse import bacc, mybir
from concourse.bass_utils import run_bass_kernel_spmd

F32 = mybir.dt.float32
BF16 = mybir.dt.bfloat16
EPS = 1e-5

N_CORES = 8
B_TOT = 16
B = B_TOT // N_CORES  # images per core
G = 4
CIN = 512
MID = 128
H = W = 56
PIX = H * W  # 3136
R = 7  # image rows per chunk
CH = R * W  # 392 pixels per chunk
NCH = H // R  # 8 chunks
SC = 4  # chunks per superchunk
NSC = NCH // SC  # 2 superchunks
HH = H + 2  # halo'd height (58)
HW2 = W + 4  # halo'd width with alignment pad (60); interior at cols 2..58
SCW = SC * CH  # pixels per superchunk (1568)

AF = mybir.ActivationFunctionType

# finals routed to path B (ACT+GPS) instead of path A (DVE): by (k, g)
PATH_B = {(0, 1), (0, 3), (1, 1)}  # per image: 3 of 8 finals via path B


def build_nc():
    # Bacc (not Bass): its compile()/finalize() pipeline legalizes sync waits
    # (>=2 waits per instruction are split into EventSemaphore instructions,
    # which this walrus build requires) and moves matmul waits to ldweights.
    nc = bacc.Bacc(None, target_bir_lowering=False)

    xs = nc.dram_tensor("xs", [B, 128, G, PIX], BF16, kind="ExternalInput")
    mup = nc.dram_tensor("mup", [B, G, PIX], BF16, kind="ExternalInput")
    mupS = nc.dram_tensor("mupS", [B, SC, G * NSC * CH], BF16, kind="ExternalInput")
    wpk = nc.dram_tensor("wpk", [128, 928], BF16, kind="ExternalInput")
    bpk = nc.dram_tensor("bpk", [128, 9], F32, kind="ExternalInput")
    ys = nc.dram_tensor("ys", [B, 128, G, PIX], BF16, kind="ExternalOutput")

    with tile.TileContext(nc) as tc:
        with (
            tc.tile_pool(name="consts", bufs=1) as consts,
            tc.tile_pool(name="xpool", bufs=2 * NSC) as xpool,
            tc.tile_pool(name="mpool", bufs=2) as mpool,
            tc.tile_pool(name="a1pool", bufs=2) as a1pool,
            tc.tile_pool(name="a2pool", bufs=8) as a2pool,
            tc.tile_pool(name="upool", bufs=2) as upool,
            tc.tile_pool(name="u2pool", bufs=4) as u2pool,
            tc.tile_pool(name="tqpool", bufs=3) as tqpool,
            tc.tile_pool(name="opool", bufs=2) as opool,
            tc.tile_pool(name="psum", bufs=1, space="PSUM") as psum,
        ):
            # ---- constants (two packed DMAs) ----
            wsb = consts.tile([128, 928], BF16)
            bsb = consts.tile([128, 9], F32)
            nc.sync.dma_start(out=wsb, in_=wpk[:])
            nc.sync.dma_start(out=bsb, in_=bpk[:])
            w1v = wsb[:, 0:128].rearrange("p (g c) -> p g c", c=32)
            w2v = wsb[:, 128:416].rearrange("p (t c) -> p t c", c=32)
            w3v = wsb[:, 416:928].rearrange("p (g c) -> p g c", c=128)
            b1v = bsb[:, 0:1]
            b2v = bsb[:, 1:5]
            b3v = bsb[:, 5:9]

            # ---- x loads (4 big DMAs, sync ring) + masks (tiny, sync ring) --
            xt = {}
            stgM = {}
            stgS = {}
            for b in range(B):
                for k in range(NSC):
                    t = xpool.tile([128, G, SCW], BF16, name=f"x_{b}_{k}", tag="x")
                    nc.sync.dma_start(
                        out=t, in_=xs[b, :, :, SCW * k : SCW * (k + 1)]
                    )
                    xt[(b, k)] = t
                    if k == 0:
                        sm = mpool.tile([128, PIX], BF16, name=f"sgM_{b}", tag="sgM")
                        for g in range(G):
                            nc.sync.dma_start(
                                out=sm[32 * g : 32 * g + 1, :],
                                in_=mup[b, g : g + 1, :],
                            )
                        ss = mpool.tile(
                            [128, G * NSC * CH], BF16, name=f"sgS_{b}", tag="sgS"
                        )
                        for j in range(SC):
                            nc.sync.dma_start(
                                out=ss[32 * j : 32 * j + 1, :],
                                in_=mupS[b, j : j + 1, :],
                            )
                        stgM[b] = sm
                        stgS[b] = ss

            # ---- PSUM: two 4-bank quads, strictly alternating ----
            qctr = [0]

            def quad(name):
                tag = f"q{qctr[0] % 2}"
                qctr[0] += 1
                return psum.tile([128, 4, 512], F32, name=name, tag=tag)

            # PE warmup: ~40 matmuls (~4.3us cold) so the HAM clock gate
            # reaches 8/8 before conv1; results are discarded.
            warm = quad("warm")
            for wi in range(40):
                nc.tensor.matmul(
                    warm[0:32, 0, :128],
                    w1v[:, 0, :],
                    w3v[:, 0, :],
                    start=True,
                    stop=True,
                    tile_position=(0, 0),
                )

            for b in range(B):
                # ---- masks: broadcast partition 32g -> whole 32-block ----
                mM = mpool.tile([128, PIX], BF16, name=f"mM_{b}", tag="mM")
                nc.vector.stream_shuffle(mM, stgM[b], [0] * 32)
                mS = mpool.tile([128, G * NSC * CH], BF16, name=f"mS_{b}", tag="mS")
                nc.vector.stream_shuffle(mS, stgS[b], [0] * 32)

                # ---- halo'd a1: zero the 1px halo border only ----
                a1h = a1pool.tile([128, HH, HW2], BF16, name=f"a1h_{b}", tag="a1h")
                nc.gpsimd.memset(a1h[:, 0, :], 0.0)
                nc.gpsimd.memset(a1h[:, HH - 1, :], 0.0)
                nc.gpsimd.memset(a1h[:, 1 : HH - 1, 0:2], 0.0)
                nc.gpsimd.memset(a1h[:, 1 : HH - 1, HW2 - 2 : HW2], 0.0)

                # ---- stage A: conv1 + relu/bias + mask -> a1h interior ----
                for q in range(NSC):
                    qA = quad(f"pA_{b}_{q}")
                    for cc in range(SC):
                        for g in range(G):
                            nc.tensor.matmul(
                                qA[32 * g : 32 * (g + 1), cc, :CH],
                                w1v[:, g, :],
                                xt[(b, q)][:, g, CH * cc : CH * (cc + 1)],
                                start=True,
                                stop=True,
                                tile_position=(0, 32 * g),
                            )
                    u1a = upool.tile([128, SC * CH], BF16, name=f"u1_{b}_{q}", tag="u1")
                    nc.scalar.activation(
                        u1a.rearrange("p (a c) -> p a c", c=CH),
                        qA[:, :, :CH],
                        AF.Relu,
                        bias=b1v,
                    )
                    nc.vector.scalar_tensor_tensor(
                        out=a1h[:, 1 + 28 * q : 1 + 28 * (q + 1), 2 : 2 + W],
                        in0=u1a.rearrange("p (r w) -> p r w", w=W),
                        scalar=0.0,
                        in1=mM[:, SCW * q : SCW * (q + 1)].rearrange(
                            "p (r w) -> p r w", w=W
                        ),
                        op0=mybir.AluOpType.add,
                        op1=mybir.AluOpType.mult,
                    )

                # ---- stages B+C per superchunk ----
                for k in range(NSC):
                    # -- B: conv2 (16-tile, 9 taps) -> quad (bank = group) --
                    qB = quad(f"pB_{b}_{k}")
                    for t in range(9):
                        ky, kx = divmod(t, 3)
                        for g in range(G):
                            for j in range(SC):
                                c = SC * k + j
                                nc.tensor.matmul(
                                    qB[32 * j : 32 * (j + 1), g, :CH],
                                    w2v[32 * g : 32 * (g + 1), t, :],
                                    a1h[
                                        32 * g : 32 * (g + 1),
                                        R * c + ky : R * c + ky + R,
                                        kx + 1 : kx + 1 + W,
                                    ],
                                    start=(t == 0),
                                    stop=(t == 8),
                                    tile_position=(32 * g, 32 * j),
                                    skip_group_check=True,
                                )
                    a2 = {}
                    for g in range(G):
                        u2 = u2pool.tile([128, CH], BF16, name=f"u2_{b}_{k}_{g}", tag="u2")
                        nc.scalar.activation(
                            u2, qB[:, g, :CH], AF.Relu, bias=b2v[:, g : g + 1]
                        )
                        at = a2pool.tile([128, CH], BF16, name=f"a2_{b}_{k}_{g}", tag="a2")
                        nc.gpsimd.tensor_mul(
                            at, u2, mS[:, (g * NSC + k) * CH : (g * NSC + k + 1) * CH]
                        )
                        a2[g] = at

                    # -- C: conv3 + (bias + residual + relu) -> ot --
                    otk = opool.tile([128, G, SCW], BF16, name=f"o_{b}_{k}", tag="o")
                    for g in range(G):
                        qC = quad(f"pC_{b}_{k}_{g}")
                        for j in range(SC):
                            nc.tensor.matmul(
                                qC[:, j, :CH],
                                w3v[32 * j : 32 * (j + 1), g, :],
                                a2[g][32 * j : 32 * (j + 1), :],
                                start=True,
                                stop=True,
                                tile_position=(32 * j, 0),
                            )
                        og = otk[:, g, :]
                        xres = xt[(b, k)][:, g, :]
                        if (k, g) in PATH_B:
                            # ACT: t = p3 + b3 ; GPS: ot = t + x ; GPS: max0
                            tq = tqpool.tile(
                                [128, SCW], BF16, name=f"t_{b}_{k}_{g}", tag="tq"
                            )
                            nc.scalar.activation(
                                tq.rearrange("p (a c) -> p a c", c=CH),
                                qC[:, :, :CH],
                                AF.Identity,
                                bias=b3v[:, g : g + 1],
                            )
                            nc.gpsimd.tensor_add(og, tq, xres)
                            nc.gpsimd.tensor_scalar_max(og, og, 0.0)
                        else:
                            # DVE: ot = (p3 + b3) + x ; DVE: max0
                            nc.vector.scalar_tensor_tensor(
                                out=og.rearrange("p (a c) -> p a c", c=CH),
                                in0=qC[:, :, :CH],
                                scalar=b3v[:, g : g + 1],
                                in1=xres.rearrange("p (a c) -> p a c", c=CH),
                                op0=mybir.AluOpType.add,
                                op1=mybir.AluOpType.add,
                            )
                            nc.vector.tensor_scalar_max(og, og, 0.0)
                    nc.scalar.dma_start(
                        out=ys[b, :, :, SCW * k : SCW * (k + 1)], in_=otk
                    )

    nc.finalize()
    return nc


def pack_params(w1, g1, b1, m1, v1, w2, g2, b2, m2, v2, w3, g3, b3, m3, v3):
    """Fold BN into weights/biases; pack for the PE mappings + const DMAs."""
    import ml_dtypes

    f32 = np.float32
    s1 = (g1 / np.sqrt(v1 + EPS)).astype(f32)
    s2 = (g2 / np.sqrt(v2 + EPS)).astype(f32)
    s3 = (g3 / np.sqrt(v3 + EPS)).astype(f32)
    c1 = (b1 - m1 * s1).astype(f32)
    c2 = (b2 - m2 * s2).astype(f32)
    c3 = (b3 - m3 * s3).astype(f32)

    w1q = w1[:, :, 0, 0].astype(f32)  # [128 out, 128 in-per-group]
    w3q = w3[:, :, 0, 0].astype(f32)  # [512 out, 32 in-per-group]

    w1l = np.zeros([128, G, 32], f32)
    for g in range(G):
        blk = w1q[32 * g : 32 * (g + 1), :] * s1[32 * g : 32 * (g + 1), None]
        w1l[:, g, :] = blk.T  # [ci=128, co=32]

    w2l = np.zeros([128, 9, 32], f32)
    for g in range(G):
        sg = s2[32 * g : 32 * (g + 1), None]
        for t in range(9):
            ky, kx = divmod(t, 3)
            blk = w2[32 * g : 32 * (g + 1), :, ky, kx].astype(f32) * sg
            w2l[32 * g : 32 * (g + 1), t, :] = blk.T  # [ci=32, co=32]

    w3l = np.zeros([128, G, 128], f32)
    for g in range(G):
        blk = (w3q[128 * g : 128 * (g + 1), :] * s3[128 * g : 128 * (g + 1), None]).T
        for j in range(4):
            w3l[32 * j : 32 * (j + 1), g, :] = blk  # [ci=32, co=128], j-replicated

    wpk = np.concatenate(
        [w1l.reshape(128, 128), w2l.reshape(128, 288), w3l.reshape(128, 512)], axis=1
    )
    bpk = np.zeros([128, 9], f32)
    bpk[:, 0] = c1
    for g in range(G):
        for j in range(4):
            bpk[32 * j : 32 * (j + 1), 1 + g] = c2[32 * g : 32 * (g + 1)]
    bpk[:, 5:9] = c3.reshape(G, 128).T
    return dict(wpk=np.ascontiguousarray(wpk.astype(ml_dtypes.bfloat16)), bpk=bpk)


def upsample_mask(mask):
    """[16, 4, 7, 7] -> bf16 ([16,4,3136] channel-major, [16,4,4*2*392] scrambled).

    mupS[b, j, g, k, p] = m[b, g, (4k+j)*CH + p] (conv2/3's chunk-scrambled view)."""
    import ml_dtypes

    m = np.repeat(np.repeat(mask, H // 7, axis=2), W // 7, axis=3)
    m = np.ascontiguousarray(m.reshape(mask.shape[0], G, PIX))
    mc = m.reshape(mask.shape[0], G, NSC, SC, CH)  # [b, g, k, j, p]
    ms = np.ascontiguousarray(mc.transpose(0, 3, 1, 2, 4))  # [b, j, g, k, p]
    ms = ms.reshape(mask.shape[0], SC, G * NSC * CH)
    return m.astype(ml_dtypes.bfloat16), ms.astype(ml_dtypes.bfloat16)


def _run(inputs, **spmd_kwargs):
    import ml_dtypes

    x = np.asarray(inputs["x"], dtype=np.float32)
    mask = np.asarray(inputs["mask"], dtype=np.float32)
    params = pack_params(
        *(np.asarray(inputs[k], dtype=np.float32)
          for k in ("w1", "g1", "b1", "m1", "v1",
                    "w2", "g2", "b2", "m2", "v2",
                    "w3", "g3", "b3", "m3", "v3"))
    )
    mup, mupS = upsample_mask(mask)
    # [B_TOT, 128, G, PIX] bf16: partition = in-channel-within-group
    xr = np.ascontiguousarray(
        x.reshape(B_TOT, G, 128, PIX).transpose(0, 2, 1, 3).astype(ml_dtypes.bfloat16)
    )

    nc = build_nc()
    in_maps = []
    for c in range(N_CORES):
        sl = slice(B * c, B * (c + 1))
        m = {
            "xs": np.ascontiguousarray(xr[sl]),
            "mup": np.ascontiguousarray(mup[sl]),
            "mupS": np.ascontiguousarray(mupS[sl]),
        }
        m.update(params)
        in_maps.append(m)

    res = run_bass_kernel_spmd(nc, in_maps, core_ids=list(range(N_CORES)), **spmd_kwargs)
    out = np.concatenate([r["ys"] for r in res.results], axis=0)
    out = out.astype(np.float32).transpose(0, 2, 1, 3)  # [B, G, 128, PIX]
    return np.ascontiguousarray(out.reshape(B_TOT, CIN, H, W)), res


def kernel(**inputs):
    out, _ = _run(inputs)
    return out


if __name__ == "__main__":
    # smoke: build only
    nc = build_nc()
    print("built ok")


# revision 5
# speedup vs baseline: 1.8670x; 1.8670x over previous
"""Trainium2 Bass kernel for nn_Bottleneck_refine (grouped bottleneck + block mask).

Reference computation (per image b):
    m   = upsample(mask[b])            # [4,7,7] -> per-group 56x56 {0,1}
    t1  = conv1x1_g4(x * m1)           # 512 -> 128, but 1x1 commutes with mask
    a1  = m . relu(s1*t1 + c1)
    t2  = conv3x3_g4(a1)               # 128 -> 128 (pad 1)
    a2  = m . relu(s2*t2 + c2)
    y   = relu(s3*conv1x1_g4(a2) + c3 + x)

Identities used:
  * m*relu(z) == relu(m*z) for m in {0,1}; 1x1 conv commutes with per-pixel
    masking, so the input mask multiply is absorbed into the relu.
  * xb := x + c3 is precomputed on host; conv1's bias is adjusted by
    -W1*c3 so conv1(xb) == conv1(x).  The final stage then needs no bias.
  * relu(p3 + xb) == xb - min(-p3, xb): the rectification becomes a min
    (DVE) plus a subtract (GPSIMD-safe; gpsimd max/relu ops are ~25x slow
    software loops on TRN2, add-family ops run at full rate).

Sharding: data-parallel over batch, 2 images per core on 8 cores.
All I/O and activations bf16: ~6.4 MB in + 6.4 MB out per core
-> memory roofline ~36us at 358 GB/s.

Layouts per image (SBUF bf16 [partition, free]):
  xt      [128, G, 1568] per superchunk k (partition = channel-in-group)
  a1h     [128, 58, 60]  halo'd masked mid activation (2-col left pad for
                         4B-aligned DVE writes)
  a2      [128, 392] per (g, k): partition 32j+co = chunk 4k+j, mid-ch co
  chunks: 7 image rows (392 px); superchunk = 4 chunks = quad of PSUM banks.

PSUM: two 4-bank quad tiles (q0 = banks 0-3, q1 = banks 4-7) alternating
between stage A chunk-quads, stage B (bank=group), stage C (bank=chunk).
Elementwise ops read whole quads (1568 px per op) to amortize the
~200-600ns fixed cost per DVE/ACT instruction.

Final stage ot = xb - min(-p3, xb), two engine paths for balance:
  path A (g even): DVE stt  t = (p3 * -1) min xb ; GPS ot = xb - t
  path B (g odd):  ACT      t = -p3 (Identity, scale=-1)
                   DVE      t = t min xb ; GPS ot = xb - t

DMA plan (sequencer dispatch is ~600ns per DMA -> few, big transfers):
sync ring: 2 const + 4 input (1.6MB each) + 4 mask (partition-strided);
scalar ring: 4 output (1.6MB each).
"""

import numpy as np

import concourse.bass as bass
import concourse.tile as tile
from concourse import bacc, mybir
from concourse.bass_utils import run_bass_kernel_spmd

F32 = mybir.dt.float32
BF16 = mybir.dt.bfloat16
EPS = 1e-5

N_CORES = 8
B_TOT = 16
B = B_TOT // N_CORES  # images per core
G = 4
CIN = 512
MID = 128
H = W = 56
PIX = H * W  # 3136
R = 7  # image rows per chunk
CH = R * W  # 392 pixels per chunk
NCH = H // R  # 8 chunks
SC = 4  # chunks per superchunk
NSC = NCH // SC  # 2 superchunks
HH = H + 2  # halo'd height (58)
HW2 = W + 4  # halo'd width with alignment pad (60); interior at cols 2..58
SCW = SC * CH  # pixels per superchunk (1568)

AF = mybir.ActivationFunctionType
ALU = mybir.AluOpType


def build_nc():
    # Bacc (not Bass): its compile()/finalize() pipeline legalizes sync waits
    # (>=2 waits per instruction are split into EventSemaphore instructions,
    # which this walrus build requires) and moves matmul waits to ldweights.
    nc = bacc.Bacc(None, target_bir_lowering=False)

    xs = nc.dram_tensor("xs", [B, 128, G, PIX], BF16, kind="ExternalInput")
    mup = nc.dram_tensor("mup", [B, G, PIX], BF16, kind="ExternalInput")
    mupS = nc.dram_tensor("mupS", [B, SC, G * NSC * CH], BF16, kind="ExternalInput")
    wpk = nc.dram_tensor("wpk", [128, 928], BF16, kind="ExternalInput")
    bpk = nc.dram_tensor("bpk", [128, 5], F32, kind="ExternalInput")
    ys = nc.dram_tensor("ys", [B, 128, G, PIX], BF16, kind="ExternalOutput")

    with tile.TileContext(nc) as tc:
        with (
            tc.tile_pool(name="consts", bufs=1) as consts,
            tc.tile_pool(name="xpool", bufs=2 * NSC) as xpool,
            tc.tile_pool(name="mpool", bufs=2) as mpool,
            tc.tile_pool(name="a1pool", bufs=2) as a1pool,
            tc.tile_pool(name="a2pool", bufs=8) as a2pool,
            tc.tile_pool(name="upool", bufs=2) as upool,
            tc.tile_pool(name="u2pool", bufs=4) as u2pool,
            tc.tile_pool(name="tqpool", bufs=4) as tqpool,
            tc.tile_pool(name="opool", bufs=2) as opool,
            tc.tile_pool(name="psum", bufs=1, space="PSUM") as psum,
        ):
            # ---- constants (two packed DMAs) ----
            wsb = consts.tile([128, 928], BF16)
            bsb = consts.tile([128, 5], F32)
            nc.sync.dma_start(out=wsb, in_=wpk[:])
            nc.sync.dma_start(out=bsb, in_=bpk[:])
            w1v = wsb[:, 0:128].rearrange("p (g c) -> p g c", c=32)
            w2v = wsb[:, 128:416].rearrange("p (t c) -> p t c", c=32)
            w3v = wsb[:, 416:928].rearrange("p (g c) -> p g c", c=128)
            b1v = bsb[:, 0:1]
            b2v = bsb[:, 1:5]

            # ---- x loads (4 big DMAs) + masks (partition-strided, 1 per
            # image per kind), all on the sync ring ----
            xt = {}
            stgM = {}
            stgS = {}
            for b in range(B):
                for k in range(NSC):
                    t = xpool.tile([128, G, SCW], BF16, name=f"x_{b}_{k}", tag="x")
                    nc.sync.dma_start(
                        out=t, in_=xs[b, :, :, SCW * k : SCW * (k + 1)]
                    )
                    xt[(b, k)] = t
                    if k == 0:
                        sm = mpool.tile([128, PIX], BF16, name=f"sgM_{b}", tag="sgM")
                        nc.sync.dma_start(
                            out=sm.rearrange("(a q) f -> a q f", q=32)[:, 0:1, :],
                            in_=mup[b, :, None, :],
                        )
                        ss = mpool.tile(
                            [128, G * NSC * CH], BF16, name=f"sgS_{b}", tag="sgS"
                        )
                        nc.sync.dma_start(
                            out=ss.rearrange("(a q) f -> a q f", q=32)[:, 0:1, :],
                            in_=mupS[b, :, None, :],
                        )
                        stgM[b] = sm
                        stgS[b] = ss

            # ---- PSUM: two 4-bank quads, strictly alternating ----
            qctr = [0]

            def quad(name):
                tag = f"q{qctr[0] % 2}"
                qctr[0] += 1
                return psum.tile([128, 4, 512], F32, name=name, tag=tag)

            # PE warmup: ~40 matmuls (~4.3us cold) so the HAM clock gate
            # reaches 8/8 before conv1; results are discarded.
            warm = quad("warm")
            for wi in range(40):
                nc.tensor.matmul(
                    warm[0:32, 0, :128],
                    w1v[:, 0, :],
                    w3v[:, 0, :],
                    start=True,
                    stop=True,
                    tile_position=(0, 0),
                )

            for b in range(B):
                # ---- masks: broadcast partition 32g -> whole 32-block ----
                mM = mpool.tile([128, PIX], BF16, name=f"mM_{b}", tag="mM")
                nc.vector.stream_shuffle(mM, stgM[b], [0] * 32)
                mS = mpool.tile([128, G * NSC * CH], BF16, name=f"mS_{b}", tag="mS")
                nc.vector.stream_shuffle(mS, stgS[b], [0] * 32)

                # ---- halo'd a1: zero the 1px halo border only ----
                a1h = a1pool.tile([128, HH, HW2], BF16, name=f"a1h_{b}", tag="a1h")
                nc.gpsimd.memset(a1h[:, 0, :], 0.0)
                nc.gpsimd.memset(a1h[:, HH - 1, :], 0.0)
                nc.gpsimd.memset(a1h[:, 1 : HH - 1, 0:2], 0.0)
                nc.gpsimd.memset(a1h[:, 1 : HH - 1, HW2 - 2 : HW2], 0.0)

                # ---- stage A: conv1 + relu/bias + mask -> a1h interior ----
                for q in range(NSC):
                    qA = quad(f"pA_{b}_{q}")
                    for cc in range(SC):
                        for g in range(G):
                            nc.tensor.matmul(
                                qA[32 * g : 32 * (g + 1), cc, :CH],
                                w1v[:, g, :],
                                xt[(b, q)][:, g, CH * cc : CH * (cc + 1)],
                                start=True,
                                stop=True,
                                tile_position=(0, 32 * g),
                            )
                    u1a = upool.tile([128, SC * CH], BF16, name=f"u1_{b}_{q}", tag="u1")
                    nc.scalar.activation(
                        u1a.rearrange("p (a c) -> p a c", c=CH),
                        qA[:, :, :CH],
                        AF.Relu,
                        bias=b1v,
                    )
                    nc.vector.scalar_tensor_tensor(
                        out=a1h[:, 1 + 28 * q : 1 + 28 * (q + 1), 2 : 2 + W],
                        in0=u1a.rearrange("p (r w) -> p r w", w=W),
                        scalar=0.0,
                        in1=mM[:, SCW * q : SCW * (q + 1)].rearrange(
                            "p (r w) -> p r w", w=W
                        ),
                        op0=ALU.add,
                        op1=ALU.mult,
                    )

                # ---- stages B+C per superchunk ----
                for k in range(NSC):
                    # -- B: conv2 (16-tile, 9 taps) -> quad (bank = group) --
                    qB = quad(f"pB_{b}_{k}")
                    for t in range(9):
                        ky, kx = divmod(t, 3)
                        for g in range(G):
                            for j in range(SC):
                                c = SC * k + j
                                nc.tensor.matmul(
                                    qB[32 * j : 32 * (j + 1), g, :CH],
                                    w2v[32 * g : 32 * (g + 1), t, :],
                                    a1h[
                                        32 * g : 32 * (g + 1),
                                        R * c + ky : R * c + ky + R,
                                        kx + 1 : kx + 1 + W,
                                    ],
                                    start=(t == 0),
                                    stop=(t == 8),
                                    tile_position=(32 * g, 32 * j),
                                    skip_group_check=True,
                                )
                    a2 = {}
                    for g in range(G):
                        u2 = u2pool.tile([128, CH], BF16, name=f"u2_{b}_{k}_{g}", tag="u2")
                        nc.scalar.activation(
                            u2, qB[:, g, :CH], AF.Relu, bias=b2v[:, g : g + 1]
                        )
                        at = a2pool.tile([128, CH], BF16, name=f"a2_{b}_{k}_{g}", tag="a2")
                        nc.gpsimd.tensor_mul(
                            at, u2, mS[:, (g * NSC + k) * CH : (g * NSC + k + 1) * CH]
                        )
                        a2[g] = at

                    # -- C: conv3, then ot = xb - min(-p3, xb) --
                    otk = opool.tile([128, G, SCW], BF16, name=f"o_{b}_{k}", tag="o")
                    for g in range(G):
                        qC = quad(f"pC_{b}_{k}_{g}")
                        for j in range(SC):
                            nc.tensor.matmul(
                                qC[:, j, :CH],
                                w3v[32 * j : 32 * (j + 1), g, :],
                                a2[g][32 * j : 32 * (j + 1), :],
                                start=True,
                                stop=True,
                                tile_position=(32 * j, 0),
                            )
                        og = otk[:, g, :]
                        xres = xt[(b, k)][:, g, :]
                        tq = tqpool.tile(
                            [128, SCW], BF16, name=f"t_{b}_{k}_{g}", tag="tq"
                        )
                        if g % 2 == 1:
                            # path B: ACT negate-copy, DVE min
                            nc.scalar.activation(
                                tq.rearrange("p (a c) -> p a c", c=CH),
                                qC[:, :, :CH],
                                AF.Identity,
                                bias=0.0,
                                scale=-1.0,
                            )
                            nc.vector.tensor_tensor(tq, tq, xres, ALU.min)
                        else:
                            # path A: DVE fused negate+min
                            nc.vector.scalar_tensor_tensor(
                                out=tq.rearrange("p (a c) -> p a c", c=CH),
                                in0=qC[:, :, :CH],
                                scalar=-1.0,
                                in1=xres.rearrange("p (a c) -> p a c", c=CH),
                                op0=ALU.mult,
                                op1=ALU.min,
                            )
                        nc.gpsimd.tensor_sub(og, xres, tq)
                    nc.scalar.dma_start(
                        out=ys[b, :, :, SCW * k : SCW * (k + 1)], in_=otk
                    )

    nc.finalize()
    return nc


def pack_params(w1, g1, b1, m1, v1, w2, g2, b2, m2, v2, w3, g3, b3, m3, v3):
    """Fold BN into weights/biases; pack for the PE mappings + const DMAs.

    Returns wpk/bpk plus c3 (folded into x on the host side; conv1's bias
    is pre-compensated by -W1_bf16 @ c3 so conv1(x + c3) == conv1(x))."""
    import ml_dtypes

    f32 = np.float32
    s1 = (g1 / np.sqrt(v1 + EPS)).astype(f32)
    s2 = (g2 / np.sqrt(v2 + EPS)).astype(f32)
    s3 = (g3 / np.sqrt(v3 + EPS)).astype(f32)
    c1 = (b1 - m1 * s1).astype(f32)
    c2 = (b2 - m2 * s2).astype(f32)
    c3 = (b3 - m3 * s3).astype(f32)

    w1q = w1[:, :, 0, 0].astype(f32)  # [128 out, 128 in-per-group]
    w3q = w3[:, :, 0, 0].astype(f32)  # [512 out, 32 in-per-group]

    w1l = np.zeros([128, G, 32], f32)
    for g in range(G):
        blk = w1q[32 * g : 32 * (g + 1), :] * s1[32 * g : 32 * (g + 1), None]
        w1l[:, g, :] = blk.T  # [ci=128, co=32]

    w2l = np.zeros([128, 9, 32], f32)
    for g in range(G):
        sg = s2[32 * g : 32 * (g + 1), None]
        for t in range(9):
            ky, kx = divmod(t, 3)
            blk = w2[32 * g : 32 * (g + 1), :, ky, kx].astype(f32) * sg
            w2l[32 * g : 32 * (g + 1), t, :] = blk.T  # [ci=32, co=32]

    w3l = np.zeros([128, G, 128], f32)
    for g in range(G):
        blk = (w3q[128 * g : 128 * (g + 1), :] * s3[128 * g : 128 * (g + 1), None]).T
        for j in range(4):
            w3l[32 * j : 32 * (j + 1), g, :] = blk  # [ci=32, co=128], j-replicated

    # conv1 bias compensation: device conv1 sees xb = x + c3, so subtract
    # delta1[32g+co] = sum_ci w1_bf16[ci, g, co] * c3[128g + ci]
    w1bf = w1l.astype(ml_dtypes.bfloat16).astype(f32)
    b1adj = c1.copy()
    for g in range(G):
        delta = w1bf[:, g, :].T @ c3[128 * g : 128 * (g + 1)]  # [32]
        b1adj[32 * g : 32 * (g + 1)] -= delta

    wpk = np.concatenate(
        [w1l.reshape(128, 128), w2l.reshape(128, 288), w3l.reshape(128, 512)], axis=1
    )
    bpk = np.zeros([128, 5], f32)
    bpk[:, 0] = b1adj
    for g in range(G):
        for j in range(4):
            bpk[32 * j : 32 * (j + 1), 1 + g] = c2[32 * g : 32 * (g + 1)]
    return (
        dict(wpk=np.ascontiguousarray(wpk.astype(ml_dtypes.bfloat16)), bpk=bpk),
        c3,
    )


def upsample_mask(mask):
    """[16, 4, 7, 7] -> bf16 ([16,4,3136] channel-major, [16,4,4*2*392] scrambled).

    mupS[b, j, g, k, p] = m[b, g, (4k+j)*CH + p] (conv2/3's chunk-scrambled view)."""
    import ml_dtypes

    m = np.repeat(np.repeat(mask, H // 7, axis=2), W // 7, axis=3)
    m = np.ascontiguousarray(m.reshape(mask.shape[0], G, PIX))
    mc = m.reshape(mask.shape[0], G, NSC, SC, CH)  # [b, g, k, j, p]
    ms = np.ascontiguousarray(mc.transpose(0, 3, 1, 2, 4))  # [b, j, g, k, p]
    ms = ms.reshape(mask.shape[0], SC, G * NSC * CH)
    return m.astype(ml_dtypes.bfloat16), ms.astype(ml_dtypes.bfloat16)


def _run(inputs, **spmd_kwargs):
    import ml_dtypes

    x = np.asarray(inputs["x"], dtype=np.float32)
    mask = np.asarray(inputs["mask"], dtype=np.float32)
    params, c3 = pack_params(
        *(np.asarray(inputs[k], dtype=np.float32)
          for k in ("w1", "g1", "b1", "m1", "v1",
                    "w2", "g2", "b2", "m2", "v2",
                    "w3", "g3", "b3", "m3", "v3"))
    )
    mup, mupS = upsample_mask(mask)
    # xb = x + c3 (residual + final bias folded); [B_TOT, 128, G, PIX] bf16
    xb = x.reshape(B_TOT, G, 128, PIX) + c3.reshape(G, 128)[None, :, :, None]
    xr = np.ascontiguousarray(
        xb.transpose(0, 2, 1, 3).astype(ml_dtypes.bfloat16)
    )

    nc = build_nc()
    in_maps = []
    for c in range(N_CORES):
        sl = slice(B * c, B * (c + 1))
        m = {
            "xs": np.ascontiguousarray(xr[sl]),
            "mup": np.ascontiguousarray(mup[sl]),
            "mupS": np.ascontiguousarray(mupS[sl]),
        }
        m.update(params)
        in_maps.append(m)

    res = run_bass_kernel_spmd(nc, in_maps, core_ids=list(range(N_CORES)), **spmd_kwargs)
    out = np.concatenate([r["ys"] for r in res.results], axis=0)
    out = out.astype(np.float32).transpose(0, 2, 1, 3)  # [B, G, 128, PIX]
    return np.ascontiguousarray(out.reshape(B_TOT, CIN, H, W)), res


def kernel(**inputs):
    out, _ = _run(inputs)
    return out


if __name__ == "__main__":
    # smoke: build only
    nc = build_nc()
    print("built ok")


# revision 13
# speedup vs baseline: 1.9801x; 1.0606x over previous
"""Trainium2 Bass kernel for nn_Bottleneck_refine (grouped bottleneck + block mask).

Reference computation (per image b):
    m   = upsample(mask[b])            # [4,7,7] -> per-group 56x56 {0,1}
    t1  = conv1x1_g4(x * m1)           # 512 -> 128, but 1x1 commutes with mask
    a1  = m . relu(s1*t1 + c1)
    t2  = conv3x3_g4(a1)               # 128 -> 128 (pad 1)
    a2  = m . relu(s2*t2 + c2)
    y   = relu(s3*conv1x1_g4(a2) + c3 + x)

Identities used:
  * m*relu(z) == relu(m*z) for m in {0,1}; 1x1 conv commutes with per-pixel
    masking, so the input mask multiply is absorbed into the relu.
  * xb := x + c3 is precomputed on host; conv1's bias is adjusted by
    -W1*c3 so conv1(xb) == conv1(x).  The final stage then needs no bias.
  * relu(p3 + xb) == xb - min(-p3, xb): the rectification becomes a min
    (DVE) plus a subtract (GPSIMD-safe; gpsimd max/relu ops are ~25x slow
    software loops on TRN2, add-family ops run at full rate).

Sharding: data-parallel over batch, 2 images per core on 8 cores.
All I/O and activations bf16: ~6.4 MB in + 6.4 MB out per core
-> memory roofline ~36us at 358 GB/s.

Layouts per image (SBUF bf16 [partition, free]):
  xt      [128, G, 1568] per superchunk k (partition = channel-in-group)
  a1h     [128, 58, 60]  halo'd masked mid activation (2-col left pad for
                         4B-aligned DVE writes)
  a2      [128, 392] per (g, k): partition 32j+co = chunk 4k+j, mid-ch co
  chunks: 7 image rows (392 px); superchunk = 4 chunks = quad of PSUM banks.

PSUM: two 4-bank quad tiles (q0 = banks 0-3, q1 = banks 4-7) alternating
between stage A chunk-quads, stage B (bank=group), stage C (bank=chunk).
Elementwise ops read whole quads (1568 px per op) to amortize the
~200-600ns fixed cost per DVE/ACT instruction.

Final stage ot = relu(p3 + xb) as a 3-op chain that keeps PSUM reads on
the fast ACT engine and bf16 SBUF ops on DVE (measured rates: ACT
~0.64ns/elem, DVE bf16 SBUF ~0.5-1.15, DVE PSUM 1.04, GPS ~2.1):
  ACT  t = p3         (Copy, PSUM -> bf16)
  DVE  u = t + xb     (tensor_add, non-inplace for 2x packing)
  DVE  ot = max(u, 0) (tensor_scalar_max, non-inplace)

DMA plan (sequencer dispatch is ~600ns per DMA -> few, big transfers):
sync ring: 2 const + 4 input (1.6MB each) + 4 mask (partition-strided);
scalar ring: 4 output (1.6MB each).
"""

import numpy as np

import concourse.bass as bass
import concourse.tile as tile
from concourse import bacc, mybir
from concourse.bass_utils import run_bass_kernel_spmd

F32 = mybir.dt.float32
BF16 = mybir.dt.bfloat16
EPS = 1e-5

N_CORES = 8
B_TOT = 16
B = B_TOT // N_CORES  # images per core
G = 4
CIN = 512
MID = 128
H = W = 56
PIX = H * W  # 3136
R = 7  # image rows per chunk
CH = R * W  # 392 pixels per chunk
NCH = H // R  # 8 chunks
SC = 4  # chunks per superchunk
NSC = NCH // SC  # 2 superchunks
HH = H + 2  # halo'd height (58)
HW2 = W + 4  # halo'd width with alignment pad (60); interior at cols 2..58
SCW = SC * CH  # pixels per superchunk (1568)

AF = mybir.ActivationFunctionType
ALU = mybir.AluOpType


def build_nc():
    # Bacc (not Bass): its compile()/finalize() pipeline legalizes sync waits
    # (>=2 waits per instruction are split into EventSemaphore instructions,
    # which this walrus build requires) and moves matmul waits to ldweights.
    nc = bacc.Bacc(None, target_bir_lowering=False)

    xs = nc.dram_tensor("xs", [B, 128, G, PIX], BF16, kind="ExternalInput")
    mup = nc.dram_tensor("mup", [B, G, PIX], BF16, kind="ExternalInput")
    mupS = nc.dram_tensor("mupS", [B, SC, G * NSC * CH], BF16, kind="ExternalInput")
    wpk = nc.dram_tensor("wpk", [128, 928], BF16, kind="ExternalInput")
    bpk = nc.dram_tensor("bpk", [128, 5], F32, kind="ExternalInput")
    ys = nc.dram_tensor("ys", [B, 128, G, PIX], BF16, kind="ExternalOutput")

    with tile.TileContext(nc) as tc:
        with (
            tc.tile_pool(name="consts", bufs=1) as consts,
            tc.tile_pool(name="xpool", bufs=2 * NSC) as xpool,
            tc.tile_pool(name="mpool", bufs=2) as mpool,
            tc.tile_pool(name="a1pool", bufs=2) as a1pool,
            tc.tile_pool(name="a2pool", bufs=8) as a2pool,
            tc.tile_pool(name="upool", bufs=2) as upool,
            tc.tile_pool(name="u2pool", bufs=4) as u2pool,
            tc.tile_pool(name="tqpool", bufs=4) as tqpool,
            tc.tile_pool(name="opool", bufs=2) as opool,
            tc.tile_pool(name="psum", bufs=1, space="PSUM") as psum,
        ):
            # ---- constants (two packed DMAs) ----
            wsb = consts.tile([128, 928], BF16)
            bsb = consts.tile([128, 5], F32)
            nc.sync.dma_start(out=wsb, in_=wpk[:])
            nc.sync.dma_start(out=bsb, in_=bpk[:])
            w1v = wsb[:, 0:128].rearrange("p (g c) -> p g c", c=32)
            w2v = wsb[:, 128:416].rearrange("p (t c) -> p t c", c=32)
            w3v = wsb[:, 416:928].rearrange("p (g c) -> p g c", c=128)
            b1v = bsb[:, 0:1]
            b2v = bsb[:, 1:5]

            # ---- x loads (half-superchunk DMAs so compute starts early) +
            # masks (partition-strided, 1 per image per kind), sync ring ----
            xt = {}
            stgM = {}
            stgS = {}
            HSC = SCW // 2
            for b in range(B):
                for k in range(NSC):
                    t = xpool.tile([128, G, SCW], BF16, name=f"x_{b}_{k}", tag="x")
                    for h in range(2):
                        nc.sync.dma_start(
                            out=t[:, :, HSC * h : HSC * (h + 1)],
                            in_=xs[
                                b, :, :, SCW * k + HSC * h : SCW * k + HSC * (h + 1)
                            ],
                        )
                    xt[(b, k)] = t
                    if k == 0:
                        sm = mpool.tile([128, PIX], BF16, name=f"sgM_{b}", tag="sgM")
                        nc.sync.dma_start(
                            out=sm.rearrange("(a q) f -> a q f", q=32)[:, 0:1, :],
                            in_=mup[b, :, None, :],
                        )
                        ss = mpool.tile(
                            [128, G * NSC * CH], BF16, name=f"sgS_{b}", tag="sgS"
                        )
                        nc.sync.dma_start(
                            out=ss.rearrange("(a q) f -> a q f", q=32)[:, 0:1, :],
                            in_=mupS[b, :, None, :],
                        )
                        stgM[b] = sm
                        stgS[b] = ss

            # ---- PSUM: two 4-bank quads, strictly alternating ----
            qctr = [0]

            def quad(name):
                tag = f"q{qctr[0] % 2}"
                qctr[0] += 1
                return psum.tile([128, 4, 512], F32, name=name, tag=tag)

            # PE warmup: ~40 matmuls (~4.3us cold) so the HAM clock gate
            # reaches 8/8 before conv1; results are discarded.
            warm = quad("warm")
            for wi in range(40):
                nc.tensor.matmul(
                    warm[0:32, 0, :128],
                    w1v[:, 0, :],
                    w3v[:, 0, :],
                    start=True,
                    stop=True,
                    tile_position=(0, 0),
                )

            mS = {}
            a1hs = {}
            a2s = {}

            def stage_A(b):
                """masks, border memsets, conv1 + relu/bias + mask -> a1h."""
                mM = mpool.tile([128, PIX], BF16, name=f"mM_{b}", tag="mM")
                nc.vector.stream_shuffle(mM, stgM[b], [0] * 32)
                ms = mpool.tile([128, G * NSC * CH], BF16, name=f"mS_{b}", tag="mS")
                nc.vector.stream_shuffle(ms, stgS[b], [0] * 32)
                mS[b] = ms

                a1h = a1pool.tile([128, HH, HW2], BF16, name=f"a1h_{b}", tag="a1h")
                nc.gpsimd.memset(a1h[:, 0, :], 0.0)
                nc.gpsimd.memset(a1h[:, HH - 1, :], 0.0)
                nc.gpsimd.memset(a1h[:, 1 : HH - 1, 0:2], 0.0)
                nc.gpsimd.memset(a1h[:, 1 : HH - 1, HW2 - 2 : HW2], 0.0)
                a1hs[b] = a1h

                for q in range(NSC):
                    qA = quad(f"pA_{b}_{q}")
                    for cc in range(SC):
                        for g in range(G):
                            nc.tensor.matmul(
                                qA[32 * g : 32 * (g + 1), cc, :CH],
                                w1v[:, g, :],
                                xt[(b, q)][:, g, CH * cc : CH * (cc + 1)],
                                start=True,
                                stop=True,
                                tile_position=(0, 32 * g),
                            )
                    u1a = upool.tile(
                        [128, SC * CH], BF16, name=f"u1_{b}_{q}", tag="u1"
                    )
                    nc.scalar.activation(
                        u1a.rearrange("p (a c) -> p a c", c=CH),
                        qA[:, :, :CH],
                        AF.Relu,
                        bias=b1v,
                    )
                    nc.vector.scalar_tensor_tensor(
                        out=a1h[:, 1 + 28 * q : 1 + 28 * (q + 1), 2 : 2 + W],
                        in0=u1a.rearrange("p (r w) -> p r w", w=W),
                        scalar=0.0,
                        in1=mM[:, SCW * q : SCW * (q + 1)].rearrange(
                            "p (r w) -> p r w", w=W
                        ),
                        op0=ALU.add,
                        op1=ALU.mult,
                    )

            def stage_B(b, k):
                """conv2 in two 8-tile half-packs (u2 of g0/g1 can start at
                the halfway point) + relu/bias + mask -> a2."""
                qB = quad(f"pB_{b}_{k}")
                for gh in range(2):
                    for t in range(9):
                        ky, kx = divmod(t, 3)
                        for g in (2 * gh, 2 * gh + 1):
                            for j in range(SC):
                                c = SC * k + j
                                nc.tensor.matmul(
                                    qB[32 * j : 32 * (j + 1), g, :CH],
                                    w2v[32 * g : 32 * (g + 1), t, :],
                                    a1hs[b][
                                        32 * g : 32 * (g + 1),
                                        R * c + ky : R * c + ky + R,
                                        kx + 1 : kx + 1 + W,
                                    ],
                                    start=(t == 0),
                                    stop=(t == 8),
                                    tile_position=(32 * g, 32 * j),
                                    skip_group_check=True,
                                )
                    for g in (2 * gh, 2 * gh + 1):
                        u2 = u2pool.tile(
                            [128, CH], BF16, name=f"u2_{b}_{k}_{g}", tag="u2"
                        )
                        nc.scalar.activation(
                            u2, qB[:, g, :CH], AF.Relu, bias=b2v[:, g : g + 1]
                        )
                        at = a2pool.tile(
                            [128, CH], BF16, name=f"a2_{b}_{k}_{g}", tag="a2"
                        )
                        nc.vector.tensor_mul(
                            at,
                            u2,
                            mS[b][:, (g * NSC + k) * CH : (g * NSC + k + 1) * CH],
                        )
                        a2s[(b, k, g)] = at

            def stage_C(b, k):
                """conv3, then ot = relu(p3 + xb) via ACT copy / add / max0;
                store each half of ot as soon as its two groups are done."""
                last = b == B - 1 and k == NSC - 1
                otk = opool.tile([128, G, SCW], BF16, name=f"o_{b}_{k}", tag="o")
                for g in range(G):
                    qC = quad(f"pC_{b}_{k}_{g}")
                    for j in range(SC):
                        nc.tensor.matmul(
                            qC[:, j, :CH],
                            w3v[32 * j : 32 * (j + 1), g, :],
                            a2s[(b, k, g)][32 * j : 32 * (j + 1), :],
                            start=True,
                            stop=True,
                            tile_position=(32 * j, 0),
                        )
                    og = otk[:, g, :]
                    xres = xt[(b, k)][:, g, :]
                    tq = tqpool.tile([128, SCW], BF16, name=f"t_{b}_{k}_{g}", tag="tq")
                    uq = tqpool.tile([128, SCW], BF16, name=f"u_{b}_{k}_{g}", tag="uq")
                    nc.scalar.activation(
                        tq.rearrange("p (a c) -> p a c", c=CH),
                        qC[:, :, :CH],
                        AF.Identity,
                        bias=0.0,
                    )
                    # DVE is ~3.5x faster than GPS per element, but GPS has
                    # slack off the critical path; keep the last superchunk
                    # all-DVE so the tail drains fast.
                    if not last and ((k == 0 and g in (1, 3)) or (k == 1 and g == 2)):
                        nc.gpsimd.tensor_add(uq, tq, xres)
                    else:
                        nc.vector.tensor_add(uq, tq, xres)
                    nc.vector.tensor_scalar_max(og, uq, 0.0)
                    if g % 2 == 1:  # store the finished half (2 groups)
                        gh = g - 1
                        nc.scalar.dma_start(
                            out=ys[b, :, gh : gh + 2, SCW * k : SCW * (k + 1)],
                            in_=otk[:, gh : gh + 2, :],
                        )

            # emission order: image 1's stage A is interleaved after image
            # 0's first superchunk so the two images' pipelines overlap.
            stage_A(0)
            stage_B(0, 0)
            stage_C(0, 0)
            stage_A(1)
            stage_B(0, 1)
            stage_C(0, 1)
            stage_B(1, 0)
            stage_C(1, 0)
            stage_B(1, 1)
            stage_C(1, 1)

    nc.finalize()
    return nc


def pack_params(w1, g1, b1, m1, v1, w2, g2, b2, m2, v2, w3, g3, b3, m3, v3):
    """Fold BN into weights/biases; pack for the PE mappings + const DMAs.

    Returns wpk/bpk plus c3 (folded into x on the host side; conv1's bias
    is pre-compensated by -W1_bf16 @ c3 so conv1(x + c3) == conv1(x))."""
    import ml_dtypes

    f32 = np.float32
    s1 = (g1 / np.sqrt(v1 + EPS)).astype(f32)
    s2 = (g2 / np.sqrt(v2 + EPS)).astype(f32)
    s3 = (g3 / np.sqrt(v3 + EPS)).astype(f32)
    c1 = (b1 - m1 * s1).astype(f32)
    c2 = (b2 - m2 * s2).astype(f32)
    c3 = (b3 - m3 * s3).astype(f32)

    w1q = w1[:, :, 0, 0].astype(f32)  # [128 out, 128 in-per-group]
    w3q = w3[:, :, 0, 0].astype(f32)  # [512 out, 32 in-per-group]

    w1l = np.zeros([128, G, 32], f32)
    for g in range(G):
        blk = w1q[32 * g : 32 * (g + 1), :] * s1[32 * g : 32 * (g + 1), None]
        w1l[:, g, :] = blk.T  # [ci=128, co=32]

    w2l = np.zeros([128, 9, 32], f32)
    for g in range(G):
        sg = s2[32 * g : 32 * (g + 1), None]
        for t in range(9):
            ky, kx = divmod(t, 3)
            blk = w2[32 * g : 32 * (g + 1), :, ky, kx].astype(f32) * sg
            w2l[32 * g : 32 * (g + 1), t, :] = blk.T  # [ci=32, co=32]

    w3l = np.zeros([128, G, 128], f32)
    for g in range(G):
        blk = (w3q[128 * g : 128 * (g + 1), :] * s3[128 * g : 128 * (g + 1), None]).T
        for j in range(4):
            w3l[32 * j : 32 * (j + 1), g, :] = blk  # [ci=32, co=128], j-replicated

    # conv1 bias compensation: device conv1 sees xb = x + c3, so subtract
    # delta1[32g+co] = sum_ci w1_bf16[ci, g, co] * c3[128g + ci]
    w1bf = w1l.astype(ml_dtypes.bfloat16).astype(f32)
    b1adj = c1.copy()
    for g in range(G):
        delta = w1bf[:, g, :].T @ c3[128 * g : 128 * (g + 1)]  # [32]
        b1adj[32 * g : 32 * (g + 1)] -= delta

    wpk = np.concatenate(
        [w1l.reshape(128, 128), w2l.reshape(128, 288), w3l.reshape(128, 512)], axis=1
    )
    bpk = np.zeros([128, 5], f32)
    bpk[:, 0] = b1adj
    for g in range(G):
        for j in range(4):
            bpk[32 * j : 32 * (j + 1), 1 + g] = c2[32 * g : 32 * (g + 1)]
    return (
        dict(wpk=np.ascontiguousarray(wpk.astype(ml_dtypes.bfloat16)), bpk=bpk),
        c3,
    )


def upsample_mask(mask):
    """[16, 4, 7, 7] -> bf16 ([16,4,3136] channel-major, [16,4,4*2*392] scrambled).

    mupS[b, j, g, k, p] = m[b, g, (4k+j)*CH + p] (conv2/3's chunk-scrambled view)."""
    import ml_dtypes

    m = np.repeat(np.repeat(mask, H // 7, axis=2), W // 7, axis=3)
    m = np.ascontiguousarray(m.reshape(mask.shape[0], G, PIX))
    mc = m.reshape(mask.shape[0], G, NSC, SC, CH)  # [b, g, k, j, p]
    ms = np.ascontiguousarray(mc.transpose(0, 3, 1, 2, 4))  # [b, j, g, k, p]
    ms = ms.reshape(mask.shape[0], SC, G * NSC * CH)
    return m.astype(ml_dtypes.bfloat16), ms.astype(ml_dtypes.bfloat16)


def _run(inputs, **spmd_kwargs):
    import ml_dtypes

    x = np.asarray(inputs["x"], dtype=np.float32)
    mask = np.asarray(inputs["mask"], dtype=np.float32)
    params, c3 = pack_params(
        *(np.asarray(inputs[k], dtype=np.float32)
          for k in ("w1", "g1", "b1", "m1", "v1",
                    "w2", "g2", "b2", "m2", "v2",
                    "w3", "g3", "b3", "m3", "v3"))
    )
    mup, mupS = upsample_mask(mask)
    # xb = x + c3 (residual + final bias folded); [B_TOT, 128, G, PIX] bf16
    xb = x.reshape(B_TOT, G, 128, PIX) + c3.reshape(G, 128)[None, :, :, None]
    xr = np.ascontiguousarray(
        xb.transpose(0, 2, 1, 3).astype(ml_dtypes.bfloat16)
    )

    nc = build_nc()
    in_maps = []
    for c in range(N_CORES):
        sl = slice(B * c, B * (c + 1))
        m = {
            "xs": np.ascontiguousarray(xr[sl]),
            "mup": np.ascontiguousarray(mup[sl]),
            "mupS": np.ascontiguousarray(mupS[sl]),
        }
        m.update(params)
        in_maps.append(m)

    res = run_bass_kernel_spmd(nc, in_maps, core_ids=list(range(N_CORES)), **spmd_kwargs)
    out = np.concatenate([r["ys"] for r in res.results], axis=0)
    out = out.astype(np.float32).transpose(0, 2, 1, 3)  # [B, G, 128, PIX]
    return np.ascontiguousarray(out.reshape(B_TOT, CIN, H, W)), res


def kernel(**inputs):
    out, _ = _run(inputs)
    return out


if __name__ == "__main__":
    # smoke: build only
    nc = build_nc()
    print("built ok")


# revision 15
# speedup vs baseline: 2.2461x; 1.1344x over previous
"""Trainium2 Bass kernel for nn_Bottleneck_refine (grouped bottleneck + block mask).

Reference computation (per image b):
    m   = upsample(mask[b])            # [4,7,7] -> per-group 56x56 {0,1}
    t1  = conv1x1_g4(x * m1)           # 512 -> 128, but 1x1 commutes with mask
    a1  = m . relu(s1*t1 + c1)
    t2  = conv3x3_g4(a1)               # 128 -> 128 (pad 1)
    a2  = m . relu(s2*t2 + c2)
    y   = relu(s3*conv1x1_g4(a2) + c3 + x)

Identities used:
  * m*relu(z) == relu(m*z) for m in {0,1}; 1x1 conv commutes with per-pixel
    masking, so the input mask multiply is absorbed into the relu.
  * xb := x + c3 is precomputed on host; conv1's bias is adjusted by
    -W1*c3 so conv1(xb) == conv1(x).  The final stage then needs no bias.
  * relu(p3 + xb) == xb - min(-p3, xb): the rectification becomes a min
    (DVE) plus a subtract (GPSIMD-safe; gpsimd max/relu ops are ~25x slow
    software loops on TRN2, add-family ops run at full rate).

Sharding: data-parallel over batch, 2 images per core on 8 cores.
All I/O and activations bf16: ~6.4 MB in + 6.4 MB out per core
-> memory roofline ~36us at 358 GB/s.

Layouts per image (SBUF bf16 [partition, free]):
  xt      [128, G, 1568] per superchunk k (partition = channel-in-group)
  a1h     [128, 58, 60]  halo'd masked mid activation (2-col left pad for
                         4B-aligned DVE writes)
  a2      [128, 392] per (g, k): partition 32j+co = chunk 4k+j, mid-ch co
  chunks: 7 image rows (392 px); superchunk = 4 chunks = quad of PSUM banks.

PSUM: two 4-bank quad tiles (q0 = banks 0-3, q1 = banks 4-7) alternating
between stage A chunk-quads, stage B (bank=group), stage C (bank=chunk).
Elementwise ops read whole quads (1568 px per op) to amortize the
~200-600ns fixed cost per DVE/ACT instruction.

Final stage ot = relu(p3 + xb) as a 3-op chain that keeps PSUM reads on
the fast ACT engine and bf16 SBUF ops on DVE (measured rates: ACT
~0.64ns/elem, DVE bf16 SBUF ~0.5-1.15, DVE PSUM 1.04, GPS ~2.1):
  ACT  t = p3         (Copy, PSUM -> bf16)
  DVE  u = t + xb     (tensor_add, non-inplace for 2x packing)
  DVE  ot = max(u, 0) (tensor_scalar_max, non-inplace)

DMA plan (sequencer dispatch is ~600ns per DMA -> few, big transfers):
sync ring: 2 const + 4 input (1.6MB each) + 4 mask (partition-strided);
scalar ring: 4 output (1.6MB each).
"""

import numpy as np

import concourse.bass as bass
import concourse.tile as tile
from concourse import bacc, mybir
from concourse.bass_utils import run_bass_kernel_spmd

F32 = mybir.dt.float32
BF16 = mybir.dt.bfloat16
EPS = 1e-5

N_CORES = 8
B_TOT = 16
B = B_TOT // N_CORES  # images per core
G = 4
CIN = 512
MID = 128
H = W = 56
PIX = H * W  # 3136
R = 7  # image rows per chunk
CH = R * W  # 392 pixels per chunk
NCH = H // R  # 8 chunks
SC = 4  # chunks per superchunk
NSC = NCH // SC  # 2 superchunks
HH = H + 2  # halo'd height (58)
HW2 = W + 4  # halo'd width with alignment pad (60); interior at cols 2..58
SCW = SC * CH  # pixels per superchunk (1568)

AF = mybir.ActivationFunctionType
ALU = mybir.AluOpType


def build_nc():
    # Bacc (not Bass): its compile()/finalize() pipeline legalizes sync waits
    # (>=2 waits per instruction are split into EventSemaphore instructions,
    # which this walrus build requires) and moves matmul waits to ldweights.
    nc = bacc.Bacc(None, target_bir_lowering=False)

    xs = nc.dram_tensor("xs", [B, 128, G, PIX], BF16, kind="ExternalInput")
    mup = nc.dram_tensor("mup", [B, G, PIX], BF16, kind="ExternalInput")
    mupS = nc.dram_tensor("mupS", [B, SC, G * NSC * CH], BF16, kind="ExternalInput")
    wpk = nc.dram_tensor("wpk", [128, 928], BF16, kind="ExternalInput")
    bpk = nc.dram_tensor("bpk", [128, 5], F32, kind="ExternalInput")
    ys = nc.dram_tensor("ys", [B, 128, G, PIX], BF16, kind="ExternalOutput")

    with tile.TileContext(nc) as tc:
        with (
            tc.tile_pool(name="consts", bufs=1) as consts,
            tc.tile_pool(name="xpool", bufs=2 * NSC) as xpool,
            tc.tile_pool(name="mpool", bufs=2) as mpool,
            tc.tile_pool(name="a1pool", bufs=2) as a1pool,
            tc.tile_pool(name="a2pool", bufs=8) as a2pool,
            tc.tile_pool(name="upool", bufs=2) as upool,
            tc.tile_pool(name="u2pool", bufs=4) as u2pool,
            tc.tile_pool(name="tqpool", bufs=4) as tqpool,
            tc.tile_pool(name="opool", bufs=2) as opool,
            tc.tile_pool(name="psum", bufs=1, space="PSUM") as psum,
        ):
            # ---- constants (two packed DMAs) ----
            wsb = consts.tile([128, 928], BF16)
            bsb = consts.tile([128, 5], F32)
            nc.sync.dma_start(out=wsb, in_=wpk[:])
            nc.sync.dma_start(out=bsb, in_=bpk[:])
            w1v = wsb[:, 0:128].rearrange("p (g c) -> p g c", c=32)
            w2v = wsb[:, 128:416].rearrange("p (t c) -> p t c", c=32)
            w3v = wsb[:, 416:928].rearrange("p (g c) -> p g c", c=128)
            b1v = bsb[:, 0:1]
            b2v = bsb[:, 1:5]

            # ---- x loads (half-superchunk DMAs so compute starts early) +
            # masks (partition-strided, 1 per image per kind), sync ring ----
            xt = {}
            stgM = {}
            stgS = {}
            HSC = SCW // 2
            for b in range(B):
                # masks first: they gate the stage-A mask multiply
                sm = mpool.tile([128, PIX], BF16, name=f"sgM_{b}", tag="sgM")
                nc.sync.dma_start(
                    out=sm.rearrange("(a q) f -> a q f", q=32)[:, 0:1, :],
                    in_=mup[b, :, None, :],
                )
                ss = mpool.tile(
                    [128, G * NSC * CH], BF16, name=f"sgS_{b}", tag="sgS"
                )
                nc.sync.dma_start(
                    out=ss.rearrange("(a q) f -> a q f", q=32)[:, 0:1, :],
                    in_=mupS[b, :, None, :],
                )
                stgM[b] = sm
                stgS[b] = ss
                for k in range(NSC):
                    t = xpool.tile([128, G, SCW], BF16, name=f"x_{b}_{k}", tag="x")
                    for h in range(2):
                        nc.sync.dma_start(
                            out=t[:, :, HSC * h : HSC * (h + 1)],
                            in_=xs[
                                b, :, :, SCW * k + HSC * h : SCW * k + HSC * (h + 1)
                            ],
                        )
                    xt[(b, k)] = t

            # ---- PSUM: two 4-bank quads, strictly alternating ----
            qctr = [0]

            def quad(name):
                tag = f"q{qctr[0] % 2}"
                qctr[0] += 1
                return psum.tile([128, 4, 512], F32, name=name, tag=tag)

            # PE warmup: ~40 matmuls (~4.3us cold) so the HAM clock gate
            # reaches 8/8 before conv1; results are discarded.
            warm = quad("warm")
            for wi in range(40):
                nc.tensor.matmul(
                    warm[0:32, 0, :128],
                    w1v[:, 0, :],
                    w3v[:, 0, :],
                    start=True,
                    stop=True,
                    tile_position=(0, 0),
                )

            mS = {}
            a1hs = {}
            a2s = {}

            def stage_A(b):
                """masks, border memsets, conv1 + relu/bias + mask -> a1h."""
                mM = mpool.tile([128, PIX], BF16, name=f"mM_{b}", tag="mM")
                nc.vector.stream_shuffle(mM, stgM[b], [0] * 32)
                ms = mpool.tile([128, G * NSC * CH], BF16, name=f"mS_{b}", tag="mS")
                nc.vector.stream_shuffle(ms, stgS[b], [0] * 32)
                mS[b] = ms

                a1h = a1pool.tile([128, HH, HW2], BF16, name=f"a1h_{b}", tag="a1h")
                nc.gpsimd.memset(a1h[:, 0, :], 0.0)
                nc.gpsimd.memset(a1h[:, HH - 1, :], 0.0)
                nc.gpsimd.memset(a1h[:, 1 : HH - 1, 0:2], 0.0)
                nc.gpsimd.memset(a1h[:, 1 : HH - 1, HW2 - 2 : HW2], 0.0)
                a1hs[b] = a1h

                for q in range(NSC):
                    qA = quad(f"pA_{b}_{q}")
                    for cc in range(SC):
                        for g in range(G):
                            nc.tensor.matmul(
                                qA[32 * g : 32 * (g + 1), cc, :CH],
                                w1v[:, g, :],
                                xt[(b, q)][:, g, CH * cc : CH * (cc + 1)],
                                start=True,
                                stop=True,
                                tile_position=(0, 32 * g),
                            )
                    u1a = upool.tile(
                        [128, SC * CH], BF16, name=f"u1_{b}_{q}", tag="u1"
                    )
                    nc.scalar.activation(
                        u1a.rearrange("p (a c) -> p a c", c=CH),
                        qA[:, :, :CH],
                        AF.Relu,
                        bias=b1v,
                    )
                    nc.vector.scalar_tensor_tensor(
                        out=a1h[:, 1 + 28 * q : 1 + 28 * (q + 1), 2 : 2 + W],
                        in0=u1a.rearrange("p (r w) -> p r w", w=W),
                        scalar=0.0,
                        in1=mM[:, SCW * q : SCW * (q + 1)].rearrange(
                            "p (r w) -> p r w", w=W
                        ),
                        op0=ALU.add,
                        op1=ALU.mult,
                    )

            def stage_B(b, k):
                """conv2 in two 8-tile half-packs (u2 of g0/g1 can start at
                the halfway point) + relu/bias + mask -> a2."""
                qB = quad(f"pB_{b}_{k}")
                for gh in range(2):
                    for t in range(9):
                        ky, kx = divmod(t, 3)
                        for g in (2 * gh, 2 * gh + 1):
                            for j in range(SC):
                                c = SC * k + j
                                nc.tensor.matmul(
                                    qB[32 * j : 32 * (j + 1), g, :CH],
                                    w2v[32 * g : 32 * (g + 1), t, :],
                                    a1hs[b][
                                        32 * g : 32 * (g + 1),
                                        R * c + ky : R * c + ky + R,
                                        kx + 1 : kx + 1 + W,
                                    ],
                                    start=(t == 0),
                                    stop=(t == 8),
                                    tile_position=(32 * g, 32 * j),
                                    skip_group_check=True,
                                )
                    for g in (2 * gh, 2 * gh + 1):
                        u2 = u2pool.tile(
                            [128, CH], BF16, name=f"u2_{b}_{k}_{g}", tag="u2"
                        )
                        nc.scalar.activation(
                            u2, qB[:, g, :CH], AF.Relu, bias=b2v[:, g : g + 1]
                        )
                        at = a2pool.tile(
                            [128, CH], BF16, name=f"a2_{b}_{k}_{g}", tag="a2"
                        )
                        nc.vector.tensor_mul(
                            at,
                            u2,
                            mS[b][:, (g * NSC + k) * CH : (g * NSC + k + 1) * CH],
                        )
                        a2s[(b, k, g)] = at

            def stage_C(b, k):
                """conv3, then ot = relu(p3 + xb) via ACT copy / add / max0;
                store each half of ot as soon as its two groups are done."""
                last = b == B - 1 and k == NSC - 1
                otk = opool.tile([128, G, SCW], BF16, name=f"o_{b}_{k}", tag="o")
                for g in range(G):
                    qC = quad(f"pC_{b}_{k}_{g}")
                    for j in range(SC):
                        nc.tensor.matmul(
                            qC[:, j, :CH],
                            w3v[32 * j : 32 * (j + 1), g, :],
                            a2s[(b, k, g)][32 * j : 32 * (j + 1), :],
                            start=True,
                            stop=True,
                            tile_position=(32 * j, 0),
                        )
                    og = otk[:, g, :]
                    xres = xt[(b, k)][:, g, :]
                    uq = tqpool.tile([128, SCW], BF16, name=f"u_{b}_{k}_{g}", tag="uq")
                    if g % 2 == 0:
                        # DVE-fused path: u = p3 + xb in one stt
                        nc.vector.scalar_tensor_tensor(
                            out=uq.rearrange("p (a c) -> p a c", c=CH),
                            in0=qC[:, :, :CH],
                            scalar=0.0,
                            in1=xres.rearrange("p (a c) -> p a c", c=CH),
                            op0=ALU.add,
                            op1=ALU.add,
                        )
                    else:
                        # ACT copy + add (GPS off the critical path; DVE on
                        # the last superchunk so the tail drains fast)
                        tq = tqpool.tile(
                            [128, SCW], BF16, name=f"t_{b}_{k}_{g}", tag="tq"
                        )
                        nc.scalar.activation(
                            tq.rearrange("p (a c) -> p a c", c=CH),
                            qC[:, :, :CH],
                            AF.Identity,
                            bias=0.0,
                        )
                        if last:
                            nc.vector.tensor_add(uq, tq, xres)
                        else:
                            nc.gpsimd.tensor_add(uq, tq, xres)
                    nc.vector.tensor_scalar_max(og, uq, 0.0)
                    if g % 2 == 1:  # store the finished half (2 groups)
                        gh = g - 1
                        nc.sync.dma_start(
                            out=ys[b, :, gh : gh + 2, SCW * k : SCW * (k + 1)],
                            in_=otk[:, gh : gh + 2, :],
                        )

            # emission order: image 1's stage A is interleaved after image
            # 0's first superchunk so the two images' pipelines overlap.
            stage_A(0)
            stage_B(0, 0)
            stage_C(0, 0)
            stage_A(1)
            stage_B(0, 1)
            stage_C(0, 1)
            stage_B(1, 0)
            stage_C(1, 0)
            stage_B(1, 1)
            stage_C(1, 1)

    nc.finalize()
    return nc


def pack_params(w1, g1, b1, m1, v1, w2, g2, b2, m2, v2, w3, g3, b3, m3, v3):
    """Fold BN into weights/biases; pack for the PE mappings + const DMAs.

    Returns wpk/bpk plus c3 (folded into x on the host side; conv1's bias
    is pre-compensated by -W1_bf16 @ c3 so conv1(x + c3) == conv1(x))."""
    import ml_dtypes

    f32 = np.float32
    s1 = (g1 / np.sqrt(v1 + EPS)).astype(f32)
    s2 = (g2 / np.sqrt(v2 + EPS)).astype(f32)
    s3 = (g3 / np.sqrt(v3 + EPS)).astype(f32)
    c1 = (b1 - m1 * s1).astype(f32)
    c2 = (b2 - m2 * s2).astype(f32)
    c3 = (b3 - m3 * s3).astype(f32)

    w1q = w1[:, :, 0, 0].astype(f32)  # [128 out, 128 in-per-group]
    w3q = w3[:, :, 0, 0].astype(f32)  # [512 out, 32 in-per-group]

    w1l = np.zeros([128, G, 32], f32)
    for g in range(G):
        blk = w1q[32 * g : 32 * (g + 1), :] * s1[32 * g : 32 * (g + 1), None]
        w1l[:, g, :] = blk.T  # [ci=128, co=32]

    w2l = np.zeros([128, 9, 32], f32)
    for g in range(G):
        sg = s2[32 * g : 32 * (g + 1), None]
        for t in range(9):
            ky, kx = divmod(t, 3)
            blk = w2[32 * g : 32 * (g + 1), :, ky, kx].astype(f32) * sg
            w2l[32 * g : 32 * (g + 1), t, :] = blk.T  # [ci=32, co=32]

    w3l = np.zeros([128, G, 128], f32)
    for g in range(G):
        blk = (w3q[128 * g : 128 * (g + 1), :] * s3[128 * g : 128 * (g + 1), None]).T
        for j in range(4):
            w3l[32 * j : 32 * (j + 1), g, :] = blk  # [ci=32, co=128], j-replicated

    # conv1 bias compensation: device conv1 sees xb = x + c3, so subtract
    # delta1[32g+co] = sum_ci w1_bf16[ci, g, co] * c3[128g + ci]
    w1bf = w1l.astype(ml_dtypes.bfloat16).astype(f32)
    b1adj = c1.copy()
    for g in range(G):
        delta = w1bf[:, g, :].T @ c3[128 * g : 128 * (g + 1)]  # [32]
        b1adj[32 * g : 32 * (g + 1)] -= delta

    wpk = np.concatenate(
        [w1l.reshape(128, 128), w2l.reshape(128, 288), w3l.reshape(128, 512)], axis=1
    )
    bpk = np.zeros([128, 5], f32)
    bpk[:, 0] = b1adj
    for g in range(G):
        for j in range(4):
            bpk[32 * j : 32 * (j + 1), 1 + g] = c2[32 * g : 32 * (g + 1)]
    return (
        dict(wpk=np.ascontiguousarray(wpk.astype(ml_dtypes.bfloat16)), bpk=bpk),
        c3,
    )


def upsample_mask(mask):
    """[16, 4, 7, 7] -> bf16 ([16,4,3136] channel-major, [16,4,4*2*392] scrambled).

    mupS[b, j, g, k, p] = m[b, g, (4k+j)*CH + p] (conv2/3's chunk-scrambled view)."""
    import ml_dtypes

    m = np.repeat(np.repeat(mask, H // 7, axis=2), W // 7, axis=3)
    m = np.ascontiguousarray(m.reshape(mask.shape[0], G, PIX))
    mc = m.reshape(mask.shape[0], G, NSC, SC, CH)  # [b, g, k, j, p]
    ms = np.ascontiguousarray(mc.transpose(0, 3, 1, 2, 4))  # [b, j, g, k, p]
    ms = ms.reshape(mask.shape[0], SC, G * NSC * CH)
    return m.astype(ml_dtypes.bfloat16), ms.astype(ml_dtypes.bfloat16)


def _run(inputs, **spmd_kwargs):
    import ml_dtypes

    x = np.asarray(inputs["x"], dtype=np.float32)
    mask = np.asarray(inputs["mask"], dtype=np.float32)
    params, c3 = pack_params(
        *(np.asarray(inputs[k], dtype=np.float32)
          for k in ("w1", "g1", "b1", "m1", "v1",
                    "w2", "g2", "b2", "m2", "v2",
                    "w3", "g3", "b3", "m3", "v3"))
    )
    mup, mupS = upsample_mask(mask)
    # xb = x + c3 (residual + final bias folded); [B_TOT, 128, G, PIX] bf16
    xb = x.reshape(B_TOT, G, 128, PIX) + c3.reshape(G, 128)[None, :, :, None]
    xr = np.ascontiguousarray(
        xb.transpose(0, 2, 1, 3).astype(ml_dtypes.bfloat16)
    )

    nc = build_nc()
    in_maps = []
    for c in range(N_CORES):
        sl = slice(B * c, B * (c + 1))
        m = {
            "xs": np.ascontiguousarray(xr[sl]),
            "mup": np.ascontiguousarray(mup[sl]),
            "mupS": np.ascontiguousarray(mupS[sl]),
        }
        m.update(params)
        in_maps.append(m)

    res = run_bass_kernel_spmd(nc, in_maps, core_ids=list(range(N_CORES)), **spmd_kwargs)
    out = np.concatenate([r["ys"] for r in res.results], axis=0)
    out = out.astype(np.float32).transpose(0, 2, 1, 3)  # [B, G, 128, PIX]
    return np.ascontiguousarray(out.reshape(B_TOT, CIN, H, W)), res


def kernel(**inputs):
    out, _ = _run(inputs)
    return out


if __name__ == "__main__":
    # smoke: build only
    nc = build_nc()
    print("built ok")


# revision 16
# speedup vs baseline: 2.3582x; 1.0499x over previous
"""Trainium2 Bass kernel for nn_Bottleneck_refine (grouped bottleneck + block mask).

Reference computation (per image b):
    m   = upsample(mask[b])            # [4,7,7] -> per-group 56x56 {0,1}
    t1  = conv1x1_g4(x * m1)           # 512 -> 128, but 1x1 commutes with mask
    a1  = m . relu(s1*t1 + c1)
    t2  = conv3x3_g4(a1)               # 128 -> 128 (pad 1)
    a2  = m . relu(s2*t2 + c2)
    y   = relu(s3*conv1x1_g4(a2) + c3 + x)

Identities used:
  * m*relu(z) == relu(m*z) for m in {0,1}; 1x1 conv commutes with per-pixel
    masking, so the input mask multiply is absorbed into the relu.
  * xb := x + c3 is precomputed on host; conv1's bias is adjusted by
    -W1*c3 so conv1(xb) == conv1(x).  The final stage then needs no bias.
  * relu(p3 + xb) == xb - min(-p3, xb): the rectification becomes a min
    (DVE) plus a subtract (GPSIMD-safe; gpsimd max/relu ops are ~25x slow
    software loops on TRN2, add-family ops run at full rate).

Sharding: data-parallel over batch, 2 images per core on 8 cores.
All I/O and activations bf16: ~6.4 MB in + 6.4 MB out per core
-> memory roofline ~36us at 358 GB/s.

Layouts per image (SBUF bf16 [partition, free]):
  xt      [128, G, 1568] per superchunk k (partition = channel-in-group)
  a1h     [128, 58, 60]  halo'd masked mid activation (2-col left pad for
                         4B-aligned DVE writes)
  a2      [128, 392] per (g, k): partition 32j+co = chunk 4k+j, mid-ch co
  chunks: 7 image rows (392 px); superchunk = 4 chunks = quad of PSUM banks.

PSUM: two 4-bank quad tiles (q0 = banks 0-3, q1 = banks 4-7) alternating
between stage A chunk-quads, stage B (bank=group), stage C (bank=chunk).
Elementwise ops read whole quads (1568 px per op) to amortize the
~200-600ns fixed cost per DVE/ACT instruction.

Final stage ot = relu(p3 + xb) as a 3-op chain that keeps PSUM reads on
the fast ACT engine and bf16 SBUF ops on DVE (measured rates: ACT
~0.64ns/elem, DVE bf16 SBUF ~0.5-1.15, DVE PSUM 1.04, GPS ~2.1):
  ACT  t = p3         (Copy, PSUM -> bf16)
  DVE  u = t + xb     (tensor_add, non-inplace for 2x packing)
  DVE  ot = max(u, 0) (tensor_scalar_max, non-inplace)

DMA plan (sequencer dispatch is ~600ns per DMA -> few, big transfers):
sync ring: 2 const + 4 input (1.6MB each) + 4 mask (partition-strided);
scalar ring: 4 output (1.6MB each).
"""

import numpy as np

import concourse.bass as bass
import concourse.tile as tile
from concourse import bacc, mybir
from concourse.bass_utils import run_bass_kernel_spmd

F32 = mybir.dt.float32
BF16 = mybir.dt.bfloat16
EPS = 1e-5

N_CORES = 8
B_TOT = 16
B = B_TOT // N_CORES  # images per core
G = 4
CIN = 512
MID = 128
H = W = 56
PIX = H * W  # 3136
R = 7  # image rows per chunk
CH = R * W  # 392 pixels per chunk
NCH = H // R  # 8 chunks
SC = 4  # chunks per superchunk
NSC = NCH // SC  # 2 superchunks
HH = H + 2  # halo'd height (58)
HW2 = W + 4  # halo'd width with alignment pad (60); interior at cols 2..58
SCW = SC * CH  # pixels per superchunk (1568)

AF = mybir.ActivationFunctionType
ALU = mybir.AluOpType


def build_nc():
    # Bacc (not Bass): its compile()/finalize() pipeline legalizes sync waits
    # (>=2 waits per instruction are split into EventSemaphore instructions,
    # which this walrus build requires) and moves matmul waits to ldweights.
    nc = bacc.Bacc(None, target_bir_lowering=False)

    xs = nc.dram_tensor("xs", [B, 128, G, PIX], BF16, kind="ExternalInput")
    mup = nc.dram_tensor("mup", [B, G, PIX], BF16, kind="ExternalInput")
    mupS = nc.dram_tensor("mupS", [B, SC, G * NSC * CH], BF16, kind="ExternalInput")
    wpk = nc.dram_tensor("wpk", [128, 928], BF16, kind="ExternalInput")
    bpk = nc.dram_tensor("bpk", [128, 5], F32, kind="ExternalInput")
    ys = nc.dram_tensor("ys", [B, 128, G, PIX], BF16, kind="ExternalOutput")

    with tile.TileContext(nc) as tc:
        with (
            tc.tile_pool(name="consts", bufs=1) as consts,
            tc.tile_pool(name="xpool", bufs=2 * NSC) as xpool,
            tc.tile_pool(name="mpool", bufs=2) as mpool,
            tc.tile_pool(name="a1pool", bufs=2) as a1pool,
            tc.tile_pool(name="a2pool", bufs=8) as a2pool,
            tc.tile_pool(name="upool", bufs=2) as upool,
            tc.tile_pool(name="u2pool", bufs=4) as u2pool,
            tc.tile_pool(name="tqpool", bufs=4) as tqpool,
            tc.tile_pool(name="opool", bufs=2) as opool,
            tc.tile_pool(name="psum", bufs=1, space="PSUM") as psum,
        ):
            # ---- constants (two packed DMAs) ----
            wsb = consts.tile([128, 928], BF16)
            bsb = consts.tile([128, 5], F32)
            nc.sync.dma_start(out=wsb, in_=wpk[:])
            nc.sync.dma_start(out=bsb, in_=bpk[:])
            w1v = wsb[:, 0:128].rearrange("p (g c) -> p g c", c=32)
            w2v = wsb[:, 128:416].rearrange("p (t c) -> p t c", c=32)
            w3v = wsb[:, 416:928].rearrange("p (g c) -> p g c", c=128)
            b1v = bsb[:, 0:1]
            b2v = bsb[:, 1:5]

            # ---- x loads (half-superchunk DMAs so compute starts early) +
            # masks (partition-strided, 1 per image per kind), sync ring ----
            xt = {}
            stgM = {}
            stgS = {}
            HSC = SCW // 2
            for b in range(B):
                # masks first: they gate the stage-A mask multiply
                sm = mpool.tile([128, PIX], BF16, name=f"sgM_{b}", tag="sgM")
                nc.sync.dma_start(
                    out=sm.rearrange("(a q) f -> a q f", q=32)[:, 0:1, :],
                    in_=mup[b, :, None, :],
                )
                ss = mpool.tile(
                    [128, G * NSC * CH], BF16, name=f"sgS_{b}", tag="sgS"
                )
                nc.sync.dma_start(
                    out=ss.rearrange("(a q) f -> a q f", q=32)[:, 0:1, :],
                    in_=mupS[b, :, None, :],
                )
                stgM[b] = sm
                stgS[b] = ss
                for k in range(NSC):
                    t = xpool.tile([128, G, SCW], BF16, name=f"x_{b}_{k}", tag="x")
                    for h in range(2):
                        nc.sync.dma_start(
                            out=t[:, :, HSC * h : HSC * (h + 1)],
                            in_=xs[
                                b, :, :, SCW * k + HSC * h : SCW * k + HSC * (h + 1)
                            ],
                        )
                    xt[(b, k)] = t

            # ---- PSUM: two 4-bank quads, strictly alternating ----
            qctr = [0]

            def quad(name):
                tag = f"q{qctr[0] % 2}"
                qctr[0] += 1
                return psum.tile([128, 4, 512], F32, name=name, tag=tag)

            # PE warmup: ~40 matmuls (~4.3us cold) so the HAM clock gate
            # reaches 8/8 before conv1; results are discarded.
            warm = quad("warm")
            for wi in range(40):
                nc.tensor.matmul(
                    warm[0:32, 0, :128],
                    w1v[:, 0, :],
                    w3v[:, 0, :],
                    start=True,
                    stop=True,
                    tile_position=(0, 0),
                )

            mS = {}
            a1hs = {}
            a2s = {}

            def stage_A(b):
                """masks, border memsets, conv1 + relu/bias + mask -> a1h."""
                mM = mpool.tile([128, PIX], BF16, name=f"mM_{b}", tag="mM")
                nc.vector.stream_shuffle(mM, stgM[b], [0] * 32)
                ms = mpool.tile([128, G * NSC * CH], BF16, name=f"mS_{b}", tag="mS")
                nc.vector.stream_shuffle(ms, stgS[b], [0] * 32)
                mS[b] = ms

                a1h = a1pool.tile([128, HH, HW2], BF16, name=f"a1h_{b}", tag="a1h")
                nc.gpsimd.memset(a1h[:, 0, :], 0.0)
                nc.gpsimd.memset(a1h[:, HH - 1, :], 0.0)
                nc.gpsimd.memset(a1h[:, 1 : HH - 1, 0:2], 0.0)
                nc.gpsimd.memset(a1h[:, 1 : HH - 1, HW2 - 2 : HW2], 0.0)
                a1hs[b] = a1h

                for q in range(NSC):
                    qA = quad(f"pA_{b}_{q}")
                    for cc in range(SC):
                        for g in range(G):
                            nc.tensor.matmul(
                                qA[32 * g : 32 * (g + 1), cc, :CH],
                                w1v[:, g, :],
                                xt[(b, q)][:, g, CH * cc : CH * (cc + 1)],
                                start=True,
                                stop=True,
                                tile_position=(0, 32 * g),
                            )
                    u1a = upool.tile(
                        [128, SC * CH], BF16, name=f"u1_{b}_{q}", tag="u1"
                    )
                    nc.scalar.activation(
                        u1a.rearrange("p (a c) -> p a c", c=CH),
                        qA[:, :, :CH],
                        AF.Relu,
                        bias=b1v,
                    )
                    nc.vector.scalar_tensor_tensor(
                        out=a1h[:, 1 + 28 * q : 1 + 28 * (q + 1), 2 : 2 + W],
                        in0=u1a.rearrange("p (r w) -> p r w", w=W),
                        scalar=0.0,
                        in1=mM[:, SCW * q : SCW * (q + 1)].rearrange(
                            "p (r w) -> p r w", w=W
                        ),
                        op0=ALU.add,
                        op1=ALU.mult,
                    )

            def stage_B(b, k):
                """conv2 in two 8-tile half-packs (u2 of g0/g1 can start at
                the halfway point) + relu/bias + mask -> a2."""
                qB = quad(f"pB_{b}_{k}")
                for gh in range(2):
                    for t in range(9):
                        ky, kx = divmod(t, 3)
                        for g in (2 * gh, 2 * gh + 1):
                            for j in range(SC):
                                c = SC * k + j
                                nc.tensor.matmul(
                                    qB[32 * j : 32 * (j + 1), g, :CH],
                                    w2v[32 * g : 32 * (g + 1), t, :],
                                    a1hs[b][
                                        32 * g : 32 * (g + 1),
                                        R * c + ky : R * c + ky + R,
                                        kx + 1 : kx + 1 + W,
                                    ],
                                    start=(t == 0),
                                    stop=(t == 8),
                                    tile_position=(32 * g, 32 * j),
                                    skip_group_check=True,
                                )
                    for g in (2 * gh, 2 * gh + 1):
                        u2 = u2pool.tile(
                            [128, CH], BF16, name=f"u2_{b}_{k}_{g}", tag="u2"
                        )
                        nc.scalar.activation(
                            u2, qB[:, g, :CH], AF.Relu, bias=b2v[:, g : g + 1]
                        )
                        at = a2pool.tile(
                            [128, CH], BF16, name=f"a2_{b}_{k}_{g}", tag="a2"
                        )
                        nc.vector.tensor_mul(
                            at,
                            u2,
                            mS[b][:, (g * NSC + k) * CH : (g * NSC + k + 1) * CH],
                        )
                        a2s[(b, k, g)] = at

            def stage_C(b, k):
                """conv3, then ot = relu(p3 + xb) via ACT copy / add / max0;
                store each half of ot as soon as its two groups are done."""
                last = b == B - 1 and k == NSC - 1
                otk = opool.tile([128, G, SCW], BF16, name=f"o_{b}_{k}", tag="o")
                for g in range(G):
                    qC = quad(f"pC_{b}_{k}_{g}")
                    for j in range(SC):
                        nc.tensor.matmul(
                            qC[:, j, :CH],
                            w3v[32 * j : 32 * (j + 1), g, :],
                            a2s[(b, k, g)][32 * j : 32 * (j + 1), :],
                            start=True,
                            stop=True,
                            tile_position=(32 * j, 0),
                        )
                    og = otk[:, g, :]
                    xres = xt[(b, k)][:, g, :]
                    uq = tqpool.tile([128, SCW], BF16, name=f"u_{b}_{k}_{g}", tag="uq")
                    if g % 2 == 0:
                        # DVE-fused path: u = p3 + xb in one stt
                        nc.vector.scalar_tensor_tensor(
                            out=uq.rearrange("p (a c) -> p a c", c=CH),
                            in0=qC[:, :, :CH],
                            scalar=0.0,
                            in1=xres.rearrange("p (a c) -> p a c", c=CH),
                            op0=ALU.add,
                            op1=ALU.add,
                        )
                    else:
                        # ACT copy + add (GPS off the critical path; DVE on
                        # the last superchunk so the tail drains fast)
                        tq = tqpool.tile(
                            [128, SCW], BF16, name=f"t_{b}_{k}_{g}", tag="tq"
                        )
                        nc.scalar.activation(
                            tq.rearrange("p (a c) -> p a c", c=CH),
                            qC[:, :, :CH],
                            AF.Identity,
                            bias=0.0,
                        )
                        if last or g == 3:
                            nc.vector.tensor_add(uq, tq, xres)
                        else:
                            nc.gpsimd.tensor_add(uq, tq, xres)
                    nc.vector.tensor_scalar_max(og, uq, 0.0)
                    if g % 2 == 1:  # store the finished half (2 groups)
                        gh = g - 1
                        nc.sync.dma_start(
                            out=ys[b, :, gh : gh + 2, SCW * k : SCW * (k + 1)],
                            in_=otk[:, gh : gh + 2, :],
                        )

            # emission order: image 1's stage A is interleaved after image
            # 0's first superchunk so the two images' pipelines overlap.
            stage_A(0)
            stage_B(0, 0)
            stage_C(0, 0)
            stage_A(1)
            stage_B(0, 1)
            stage_C(0, 1)
            stage_B(1, 0)
            stage_C(1, 0)
            stage_B(1, 1)
            stage_C(1, 1)

    nc.finalize()
    return nc


def pack_params(w1, g1, b1, m1, v1, w2, g2, b2, m2, v2, w3, g3, b3, m3, v3):
    """Fold BN into weights/biases; pack for the PE mappings + const DMAs.

    Returns wpk/bpk plus c3 (folded into x on the host side; conv1's bias
    is pre-compensated by -W1_bf16 @ c3 so conv1(x + c3) == conv1(x))."""
    import ml_dtypes

    f32 = np.float32
    s1 = (g1 / np.sqrt(v1 + EPS)).astype(f32)
    s2 = (g2 / np.sqrt(v2 + EPS)).astype(f32)
    s3 = (g3 / np.sqrt(v3 + EPS)).astype(f32)
    c1 = (b1 - m1 * s1).astype(f32)
    c2 = (b2 - m2 * s2).astype(f32)
    c3 = (b3 - m3 * s3).astype(f32)

    w1q = w1[:, :, 0, 0].astype(f32)  # [128 out, 128 in-per-group]
    w3q = w3[:, :, 0, 0].astype(f32)  # [512 out, 32 in-per-group]

    w1l = np.zeros([128, G, 32], f32)
    for g in range(G):
        blk = w1q[32 * g : 32 * (g + 1), :] * s1[32 * g : 32 * (g + 1), None]
        w1l[:, g, :] = blk.T  # [ci=128, co=32]

    w2l = np.zeros([128, 9, 32], f32)
    for g in range(G):
        sg = s2[32 * g : 32 * (g + 1), None]
        for t in range(9):
            ky, kx = divmod(t, 3)
            blk = w2[32 * g : 32 * (g + 1), :, ky, kx].astype(f32) * sg
            w2l[32 * g : 32 * (g + 1), t, :] = blk.T  # [ci=32, co=32]

    w3l = np.zeros([128, G, 128], f32)
    for g in range(G):
        blk = (w3q[128 * g : 128 * (g + 1), :] * s3[128 * g : 128 * (g + 1), None]).T
        for j in range(4):
            w3l[32 * j : 32 * (j + 1), g, :] = blk  # [ci=32, co=128], j-replicated

    # conv1 bias compensation: device conv1 sees xb = x + c3, so subtract
    # delta1[32g+co] = sum_ci w1_bf16[ci, g, co] * c3[128g + ci]
    w1bf = w1l.astype(ml_dtypes.bfloat16).astype(f32)
    b1adj = c1.copy()
    for g in range(G):
        delta = w1bf[:, g, :].T @ c3[128 * g : 128 * (g + 1)]  # [32]
        b1adj[32 * g : 32 * (g + 1)] -= delta

    wpk = np.concatenate(
        [w1l.reshape(128, 128), w2l.reshape(128, 288), w3l.reshape(128, 512)], axis=1
    )
    bpk = np.zeros([128, 5], f32)
    bpk[:, 0] = b1adj
    for g in range(G):
        for j in range(4):
            bpk[32 * j : 32 * (j + 1), 1 + g] = c2[32 * g : 32 * (g + 1)]
    return (
        dict(wpk=np.ascontiguousarray(wpk.astype(ml_dtypes.bfloat16)), bpk=bpk),
        c3,
    )


def upsample_mask(mask):
    """[16, 4, 7, 7] -> bf16 ([16,4,3136] channel-major, [16,4,4*2*392] scrambled).

    mupS[b, j, g, k, p] = m[b, g, (4k+j)*CH + p] (conv2/3's chunk-scrambled view)."""
    import ml_dtypes

    m = np.repeat(np.repeat(mask, H // 7, axis=2), W // 7, axis=3)
    m = np.ascontiguousarray(m.reshape(mask.shape[0], G, PIX))
    mc = m.reshape(mask.shape[0], G, NSC, SC, CH)  # [b, g, k, j, p]
    ms = np.ascontiguousarray(mc.transpose(0, 3, 1, 2, 4))  # [b, j, g, k, p]
    ms = ms.reshape(mask.shape[0], SC, G * NSC * CH)
    return m.astype(ml_dtypes.bfloat16), ms.astype(ml_dtypes.bfloat16)


def _run(inputs, **spmd_kwargs):
    import ml_dtypes

    x = np.asarray(inputs["x"], dtype=np.float32)
    mask = np.asarray(inputs["mask"], dtype=np.float32)
    params, c3 = pack_params(
        *(np.asarray(inputs[k], dtype=np.float32)
          for k in ("w1", "g1", "b1", "m1", "v1",
                    "w2", "g2", "b2", "m2", "v2",
                    "w3", "g3", "b3", "m3", "v3"))
    )
    mup, mupS = upsample_mask(mask)
    # xb = x + c3 (residual + final bias folded); [B_TOT, 128, G, PIX] bf16
    xb = x.reshape(B_TOT, G, 128, PIX) + c3.reshape(G, 128)[None, :, :, None]
    xr = np.ascontiguousarray(
        xb.transpose(0, 2, 1, 3).astype(ml_dtypes.bfloat16)
    )

    nc = build_nc()
    in_maps = []
    for c in range(N_CORES):
        sl = slice(B * c, B * (c + 1))
        m = {
            "xs": np.ascontiguousarray(xr[sl]),
            "mup": np.ascontiguousarray(mup[sl]),
            "mupS": np.ascontiguousarray(mupS[sl]),
        }
        m.update(params)
        in_maps.append(m)

    res = run_bass_kernel_spmd(nc, in_maps, core_ids=list(range(N_CORES)), **spmd_kwargs)
    out = np.concatenate([r["ys"] for r in res.results], axis=0)
    out = out.astype(np.float32).transpose(0, 2, 1, 3)  # [B, G, 128, PIX]
    return np.ascontiguousarray(out.reshape(B_TOT, CIN, H, W)), res


def kernel(**inputs):
    out, _ = _run(inputs)
    return out


if __name__ == "__main__":
    # smoke: build only
    nc = build_nc()
    print("built ok")
